# revision 105
# baseline (speedup 1.0000x reference)
"""Causal self-attention (B=2, T=2048, C=1024, 16 heads) on 8 Trainium2 cores.

Sharding: core = b*4 + g. Each core handles batch b and heads [4g, 4g+4)
(256 of the 1024 channel dims). It computes q/k/v for its heads, causal
attention, and the c_proj partial product against the matching 256-row slice
of w_proj. The host sums the 4 per-core partials of each batch (the
all-reduce after c_proj, done for free on the host).

Per-core bass kernel (bf16 matmul operands, fp32 PSUM accumulation):
  qkv:   per t-block of 512, wqk/x chunk matmuls into PSUM, PSUM->SBUF
         copies (cast to bf16) on DVE (GPSIMD cannot access PSUM).
  attn:  per (qi, pc=head-pair, jb): S^T tiles [j=128, q<=512] on PE,
         exp on ACT (scale fused), diagonal-band mask on DVE. P@V runs
         with the probability tile STATIONARY: out po[q=128, 65] per
         (head, q-subblock), accumulated over jb (65 matmul rows per
         j-block instead of q-width -- half the PE cost of the [d, q]
         orientation); the 65th column (ones appended to v) gives the
         softmax denominator per out-PARTITION, so normalization is one
         reciprocal + per-partition tensor_scalar mul on DVE. po packs
         2 q-subblocks x 2 heads per PSUM bank (one accumulation zero-
         region each; start on first touch, stop on the bank's last
         write). A PE transpose (identity matmul) flips normalized o to
         [d, q] for c_proj.
  proj:  y[t,e] accumulated over the two 128-row d' chunks, bf16 out,
         DMA'd per [128, 512] tile.

Emission is software-pipelined: the exp stream on ACT (~1.1us/j-block)
is slower than the attention matmuls on PE (~0.65us), so QKV for
t-blocks 1..3, the o transposes, and c_proj tiles are woven between
attention groups (~KFILL ns of filler per j-block, plus per-unit due
slots just before their first consumer). The TileContext scheduler is
dataflow-based; generous work-pool bufs (e/osb/rden/ys) are essential so
its schedule can decouple the streams, and DMA-gated units carry
tile_wait_until lower bounds.
"""

import numpy as np
import ml_dtypes

import concourse.bass as bass
import concourse.tile as tile
from concourse import bacc, masks, mybir
from concourse.bass_utils import run_bass_kernel_spmd

B, T, C = 2, 2048, 1024
NH, HD = 16, 64
NCORES = 8
GROUPS = 4              # head-groups; cores per batch
HPC = NH // GROUPS      # 4 heads per core
DQ = HPC * HD           # 256 head-dims per core
P = 128
CCH = C // P            # 8 contraction chunks over C
QB = 512                # q-block (free dim of S^T tiles)
NQB = T // QB           # 4
NJB = T // P            # 16 j-blocks / t-blocks of 128
EB = 512                # proj output block
BF = mybir.dt.bfloat16
F32 = mybir.dt.float32
EXP = mybir.ActivationFunctionType.Exp

# ---- virtual-clock constants (ns), used only to order emission ----
PE_C = 1.0 / 2.4        # ns per matmul row at peak
ACT_C = 1.0 / 1.2
DVE_C = 1.0 / 0.96
POOL_C = 1.0 / (1.2 * 0.6)
SEM = 190.0
ACT_OVH = 300.0
DVE_OVH = 200.0
POOL_OVH = 140.0
DMA_B = 1.0 / 360.0     # ns per byte at 360GB/s aggregate

_PROGRAM = None
LAST_RESULTS = None     # test.py reads profiling info from here


def _build_program():
    nc = bacc.Bacc("TRN2", target_bir_lowering=False, debug=False)

    xt_d = nc.dram_tensor("xt", [C, T], BF, kind="ExternalInput")
    wqk_d = nc.dram_tensor("wqk", [C, 2 * DQ], BF, kind="ExternalInput")
    wv_d = nc.dram_tensor("wv", [C, DQ], BF, kind="ExternalInput")
    wp_d = nc.dram_tensor("wp", [DQ, C], BF, kind="ExternalInput")
    msk_d = nc.dram_tensor("msk", [P, P], BF, kind="ExternalInput")
    y_d = nc.dram_tensor("y", [T, C], BF, kind="ExternalOutput")

    clk = {"pe": 0.0, "act": 0.0, "dve": 0.0, "pool": 0.0, "dma": 300.0}

    def pe_adv(rows):
        clk["pe"] += rows * PE_C

    def dma_adv(bytes_total):
        clk["dma"] += bytes_total * DMA_B
        return clk["dma"] + 900.0   # + sem propagation

    with tile.TileContext(nc) as tc:
        with (
            tc.tile_pool(name="persist", bufs=1) as persist,
            tc.tile_pool(name="work", bufs=4) as work,
            tc.tile_pool(name="ps", bufs=1, space="PSUM") as ps,
        ):
            # ---- persistent SBUF tensors ----
            msk = persist.tile([P, P], BF)
            idn = persist.tile([P, P], BF)
            xT = persist.tile([P, CCH, T], BF)             # x[b].T  (c, t)
            wqk = persist.tile([P, CCH, 2 * DQ], BF)       # [wq.T | wk.T]
            wv = persist.tile([P, CCH, DQ], BF)
            wp = persist.tile([P, 2, C], BF)
            qT = [persist.tile([P, T], BF, name=f"qT{i}") for i in range(2)]
            kT = [persist.tile([P, T], BF, name=f"kT{i}") for i in range(2)]
            vv = [persist.tile([P, NJB, 2, HD + 1], BF, name=f"vv{i}")
                  for i in range(2)]
            oTs = [persist.tile([P, NJB, P], BF, name=f"oTs{i}")
                   for i in range(2)]

            # ---- input DMAs (scalar queue), fine at head, coarse later ----
            xr = xt_d.ap().rearrange("(o p) f -> p o f", p=P)
            wqk_r = wqk_d.ap().rearrange("(o p) f -> p o f", p=P)
            wv_r = wv_d.ap().rearrange("(o p) f -> p o f", p=P)
            wp_r = wp_d.ap().rearrange("(c p) e -> p c e", p=P)

            x_avail = np.zeros((NQB, CCH))  # est. land time of x chunks
            w_avail = np.zeros(CCH)

            def dma_w_chunks(c0, c1):
                nc.scalar.dma_start(wqk[:, c0:c1, :], wqk_r[:, c0:c1, :])
                t = dma_adv((c1 - c0) * P * 2 * DQ * 2)
                w_avail[c0:c1] = t

            def dma_x_chunks(tb, c0, c1):
                sl = slice(tb * QB, (tb + 1) * QB)
                nc.scalar.dma_start(xT[:, c0:c1, sl], xr[:, c0:c1, sl])
                t = dma_adv((c1 - c0) * P * QB * 2)
                x_avail[tb, c0:c1] = t

            dma_w_chunks(0, 1)
            dma_x_chunks(0, 0, 1)
            dma_w_chunks(1, 2)
            dma_x_chunks(0, 1, 2)
            nc.scalar.dma_start(msk[:], msk_d.ap())
            dma_adv(P * P * 2 * 2)          # sub-512B rows: 2x multiplier
            dma_w_chunks(2, 4)
            dma_x_chunks(0, 2, 4)
            dma_w_chunks(4, 8)
            dma_x_chunks(0, 4, 6)
            nc.scalar.dma_start(wv[:, 0:4, :], wv_r[:, 0:4, :])
            dma_adv(P * 4 * DQ * 2)
            dma_x_chunks(0, 6, 8)
            nc.scalar.dma_start(wv[:, 4:8, :], wv_r[:, 4:8, :])
            wv_avail = dma_adv(P * 4 * DQ * 2)
            dma_x_chunks(1, 0, 8)
            nc.scalar.dma_start(wp[:], wp_r[:])
            wp_avail = dma_adv(P * 2 * C * 2)
            dma_x_chunks(2, 0, 8)
            dma_x_chunks(3, 0, 8)

            # ones column of v tiles; identity for the PE transpose
            for i in range(2):
                nc.vector.memset(vv[i][:, :, :, HD:HD + 1], 1.0)
            masks.make_identity(nc, idn[:])

            # ================= filler units =================
            # qkv units for tb>=1 + transposes + proj tiles, woven into the
            # attention stream whenever PE would idle.
            # Each DMA-gated unit is emitted under tile_wait_until(<its DMA
            # land estimate>): the scheduling sim does not model DMA-
            # bandwidth contention, so without the time pin it hoists such
            # work (and its PSUM->SBUF copies) far ahead of the attention
            # chain, head-of-line-blocking it at runtime. The pin must be a
            # LOWER bound on real readiness -- it is honored by the runtime
            # too, so an inflated value would stall the in-order PE queue.
            def pin(ts):
                return tc.tile_wait_until(ts / 1e6)
            timed_units = []    # qkv fillers with a due sequence slot
            tp_units = []       # transposes (priority)
            proj_units = []     # proj tiles
            oTs_done = [[0.0] * NJB for _ in range(2)]
            ys_pend = {}        # Q -> staged [P, 2, EB] y tile

            def emit_qk(tb, pc, which):
                # one of q/k for head-pair pc of t-block tb: 8 chunk matmuls
                sl = slice(tb * QB, (tb + 1) * QB)
                col = pc * P if which == 0 else DQ + pc * P
                with pin(max(x_avail[tb, 7], PIN_W * clk["pe"])):
                    pq = ps.tile([P, QB], F32, tag="out", bufs=2, name="pq")
                    for c in range(CCH):
                        clk["pe"] = max(clk["pe"], x_avail[tb, c], w_avail[c])
                        nc.tensor.matmul(
                            pq[:], wqk[:, c, col:col + P], xT[:, c, sl],
                            start=(c == 0), stop=(c == CCH - 1),
                        )
                        pe_adv(QB)
                    dst = qT[pc] if which == 0 else kT[pc]
                    clk["dve"] = max(clk["dve"], clk["pe"] + SEM)
                    nc.vector.tensor_copy(dst[:, sl], pq[:])
                    clk["dve"] += QB * DVE_C + DVE_OVH

            def emit_v(tb, tt):
                with pin(max(x_avail[tb, 7], wv_avail, PIN_W * clk["pe"])):
                    pv = ps.tile([P, DQ], F32, tag="out", bufs=2, name="pv")
                    for c in range(CCH):
                        clk["pe"] = max(clk["pe"], x_avail[tb, c], wv_avail)
                        nc.tensor.matmul(
                            pv[:], xT[:, c, tt * P:(tt + 1) * P], wv[:, c, :],
                            start=(c == 0), stop=(c == CCH - 1),
                        )
                        pe_adv(DQ)
                    for pc in range(2):
                        clk["dve"] = max(clk["dve"], clk["pe"] + SEM)
                        nc.vector.tensor_copy(
                            vv[pc][:, tt, :, 0:HD],
                            pv[:, pc * P:(pc + 1) * P].rearrange(
                                "p (h d) -> p h d", d=HD),
                        )
                        clk["dve"] += 2 * HD * DVE_C + DVE_OVH

            # global attention-iteration sequence numbers: the (qi, pc, jb)
            # loop below advances `seq` by 1 per j-block. qkv units are due
            # spread over the iterations just before their consumer.
            def seq_of(qi, pc, jb):
                return sum(2 * (4 * q + 4) for q in range(qi)) \
                    + pc * (4 * qi + 4) + jb

            for tb in range(1, NQB):
                for pc in range(2):
                    # qT[pc] of tb first read at S(qi=tb, pc, jb=0);
                    # kT[pc] of tb first read at S(qi=tb, pc, jb=4tb)
                    timed_units.append(dict(
                        due=seq_of(tb, pc, 0) - 2,
                        ready=x_avail[tb, 7],
                        emit=(lambda tb=tb, pc=pc: emit_qk(tb, pc, 0)),
                    ))
                    timed_units.append(dict(
                        due=seq_of(tb, pc, 4 * tb) - 2,
                        ready=x_avail[tb, 7],
                        emit=(lambda tb=tb, pc=pc: emit_qk(tb, pc, 1)),
                    ))
                for tt in range(4 * tb, 4 * tb + 4):
                    # vv[tt] first read at PV(qi=tb, pc=0, jb=tt)
                    timed_units.append(dict(
                        due=seq_of(tb, 0, tt) - 2,
                        ready=x_avail[tb, 7],
                        emit=(lambda tb=tb, tt=tt: emit_v(tb, tt)),
                    ))
            timed_units.sort(key=lambda u: u["due"])

            def emit_transpose(pc, Q, osb, ready_dve):
                clk["pe"] = max(clk["pe"], ready_dve)
                ot = ps.tile([P, P], BF, tag="out", bufs=2, name="ot")
                nc.tensor.transpose(ot[:], osb[:, :, :], idn[:])
                pe_adv(P)
                clk["dve"] = max(clk["dve"], clk["pe"] + SEM)
                nc.vector.tensor_copy(oTs[pc][:, Q, :], ot[:])
                clk["dve"] += P * DVE_C + DVE_OVH
                oTs_done[pc][Q] = clk["dve"] + SEM
                if pc == 1:
                    rdy = max(oTs_done[0][Q], oTs_done[1][Q], wp_avail)
                    for eb in range(C // EB):
                        # due slots spread proj work toward the late blocks,
                        # where qkv filler supply has run dry
                        proj_units.append(dict(
                            ready=rdy,
                            due=16 + 4 * Q + 2 * eb,
                            emit=(lambda Q=Q, eb=eb: emit_proj(Q, eb)),
                        ))

            def emit_proj(Q, eb):
                sl = slice(eb * EB, (eb + 1) * EB)
                with pin(max(wp_avail, PIN_W * clk["pe"])):
                    # tail tiles borrow the st banks (free after last exp)
                    # to deepen the psum rotation
                    ytag = "st" if Q >= 13 and eb == 0 else "out"
                    yp = ps.tile([P, EB], F32, tag=ytag, bufs=2, name="yp")
                    nc.tensor.matmul(yp[:], oTs[0][:, Q, :], wp[:, 0, sl],
                                     start=True, stop=False)
                    nc.tensor.matmul(yp[:], oTs[1][:, Q, :], wp[:, 1, sl],
                                     start=False, stop=True)
                    pe_adv(2 * EB)
                    if eb == 0:
                        ys_pend[Q] = work.tile([P, 2, EB], BF, tag="ys",
                                               bufs=8, name="ys")
                    ys = ys_pend[Q]
                    if Q >= 8 and eb == 1:
                        # tail: the exp stream is done, ACT is free
                        clk["act"] = max(clk["act"], clk["pe"] + SEM)
                        clk["act"] += EB * ACT_C + ACT_OVH
                        nc.scalar.copy(ys[:, eb, :], yp[:])
                    else:
                        clk["dve"] = max(clk["dve"], clk["pe"] + SEM)
                        clk["dve"] += EB * DVE_C + DVE_OVH
                        nc.vector.tensor_copy(ys[:, eb, :], yp[:])
                    if eb == 1:
                        # one merged [P, 1024] DMA per t-row block (HWDGE
                        # issue overhead dominates small DMAs)
                        nc.sync.dma_start(
                            y_d.ap()[Q * P:(Q + 1) * P, :], ys[:, :, :])

            # Fixed-rate weave: attention is paced by the ACT exp stream at
            # ~1.1us per j-block while its PE work is only ~0.65us, so emit
            # ~FILL ns of filler work per iteration, unconditionally.
            FILL = float(__import__("os").environ.get("KFILL", "460"))
            PIN_W = float(__import__("os").environ.get("KPINW", "0.85"))
            fill_state = dict(emitted=0.0, quota=0.0)

            def step_fillers(extra=0.0):
                fill_state["quota"] += FILL + extra
                while fill_state["emitted"] < fill_state["quota"]:
                    best = None
                    for lst in (tp_units, timed_units, proj_units):
                        if (lst is proj_units and lst
                                and lst[0].get("due", 0) >
                                fill_state.get("seq", 1 << 30)):
                            continue
                        if lst and lst[0]["ready"] <= clk["pe"] + 300.0 and (
                                best is None
                                or lst[0]["ready"] < best[0]["ready"]):
                            best = lst
                    if best is None:
                        break
                    u = best.pop(0)
                    t0 = clk["pe"]
                    clk["pe"] = max(clk["pe"], u["ready"])
                    u["emit"]()
                    fill_state["emitted"] += clk["pe"] - t0

            # ---- t-block 0 qkv, chunk-major (DMA-paced) ----
            pqk0 = [ps.tile([P, 2, QB], F32, tag="st", bufs=2, name=f"pqk{pc}")
                    for pc in range(2)]
            for c in range(CCH):
                clk["pe"] = max(clk["pe"], x_avail[0, c], w_avail[c])
                with pin(max(x_avail[0, c], w_avail[c])):
                    for pc in range(2):
                        nc.tensor.matmul(
                            pqk0[pc][:, 0, :], wqk[:, c, pc * P:(pc + 1) * P],
                            xT[:, c, 0:QB],
                            start=(c == 0), stop=(c == CCH - 1))
                        nc.tensor.matmul(
                            pqk0[pc][:, 1, :],
                            wqk[:, c, DQ + pc * P:DQ + (pc + 1) * P],
                            xT[:, c, 0:QB],
                            start=(c == 0), stop=(c == CCH - 1))
                        pe_adv(2 * QB)
            for pc in range(2):
                clk["dve"] = max(clk["dve"], clk["pe"] + SEM)
                nc.vector.tensor_copy(qT[pc][:, 0:QB], pqk0[pc][:, 0, :])
                nc.vector.tensor_copy(kT[pc][:, 0:QB], pqk0[pc][:, 1, :])
                clk["dve"] += 2 * (QB * DVE_C + DVE_OVH)
            for tt in range(4):
                emit_v(0, tt)

            # ---- attention + woven fillers ----
            seq = 0
            for qi in range(NQB):
                for pc in range(2):
                    njb = 4 * qi + 4
                    # [pair, ti%2, h, 65] padded so each ti-pair owns one
                    # PSUM bank (= accumulation zero-region): one start/stop
                    # group per bank, start on first touch, stop on the
                    # bank's last write (odd ti's diagonal, h=1).
                    po = ps.tile([P, 2, 2, 2, HD + 1], F32, tag="po", bufs=1,
                                 padded_shape=[P, 2, 2, 2, P], name="po")
                    bank_started = [False, False]

                    def pv_mm(h, ti, e_ap, jb):
                        pair, tih = divmod(ti, 2)
                        start = (jb == 0) and not bank_started[pair]
                        if start:
                            bank_started[pair] = True
                        stop = (ti == jb - 4 * qi) and (ti % 2 == 1) and h == 1
                        nc.tensor.matmul(
                            po[:, pair, tih, h, :], e_ap,
                            vv[pc][:, jb, h, :],
                            start=start, stop=stop,
                        )
                        pe_adv(HD + 1)

                    for jb in range(njb):
                        t = jb - 4 * qi
                        lo = P * t if t > 0 else 0
                        width = QB - lo
                        fill_state["seq"] = seq
                        while timed_units and timed_units[0]["due"] <= seq:
                            u = timed_units.pop(0)
                            clk["pe"] = max(clk["pe"], u["ready"])
                            u["emit"]()
                        seq += 1
                        st = ps.tile([P, 2, QB], F32, tag="st", bufs=2,
                                     name="st")
                        for h in range(2):
                            nc.tensor.matmul(
                                st[:, h, lo:QB],
                                kT[pc][h * HD:(h + 1) * HD, jb * P:(jb + 1) * P],
                                qT[pc][h * HD:(h + 1) * HD,
                                       qi * QB + lo:(qi + 1) * QB],
                                start=True, stop=True,
                            )
                            pe_adv(width)
                        s_done = clk["pe"]
                        e = work.tile([P, 2, QB], BF, tag="e", bufs=40,
                                      name="e")
                        clk["act"] = max(clk["act"], s_done + SEM)
                        clk["act"] += 2 * width * ACT_C + ACT_OVH
                        nc.scalar.activation(e[:, :, lo:QB], st[:, :, lo:QB],
                                             EXP, scale=0.125)
                        if t >= 0:
                            clk["dve"] = max(clk["dve"], clk["act"] + SEM)
                            clk["dve"] += 2 * P * DVE_C + DVE_OVH
                            nc.vector.tensor_mul(
                                e[:, :, lo:lo + P], e[:, :, lo:lo + P],
                                msk[:, None, 0:P].to_broadcast((P, 2, P)),
                            )
                        # fillers land between exp and the P@V consumers so
                        # PE stays busy while ACT/DVE produce e
                        step_fillers(150.0 if t >= 0 else 0.0)
                        # P@V: probability tile stationary, v+ones moving.
                        # Unmasked q-subblocks first; the masked diagonal one
                        # last to cover the DVE mask latency.
                        for ti in range(max(t, 0) + 1 if t >= 0 else 0, 4):
                            for h in range(2):
                                pv_mm(h, ti, e[:, h, ti * P:(ti + 1) * P], jb)
                        if t >= 0:
                            for h in range(2):
                                pv_mm(h, t, e[:, h, lo:lo + P], jb)
                        if t >= 0 and t % 2 == 1:
                            # bank (pair t//2) group closed: normalize both
                            # of its q-subblocks on DVE
                            pair = t // 2
                            rden = work.tile([P, 2, 2, 1], F32, tag="rden",
                                             bufs=96, name="rden")
                            clk["dve"] = max(clk["dve"], clk["pe"] + SEM)
                            nc.vector.reciprocal(
                                rden[:], po[:, pair, :, :, HD:HD + 1])
                            clk["dve"] += 4 * DVE_C + DVE_OVH + 120
                            for tih in range(2):
                                osb = work.tile([P, 2, HD], BF, tag="osb",
                                                bufs=96, name="osb")
                                for h in range(2):
                                    nc.vector.tensor_scalar_mul(
                                        osb[:, h, :],
                                        po[:, pair, tih, h, 0:HD],
                                        rden[:, tih, h, :])
                                    clk["dve"] += HD * DVE_C + DVE_OVH
                                tp_units.append(dict(
                                    ready=clk["dve"] + SEM,
                                    emit=(lambda pc=pc,
                                          Q=4 * qi + 2 * pair + tih, osb=osb,
                                          r=clk["dve"] + SEM:
                                          emit_transpose(pc, Q, osb, r)),
                                ))

            # ---- tail: whatever fillers remain ----
            for lst in (tp_units, timed_units, proj_units):
                while lst:
                    u = lst.pop(0)
                    clk["pe"] = max(clk["pe"], u["ready"])
                    u["emit"]()

    nc.compile()
    return nc


def _get_program():
    global _PROGRAM
    if _PROGRAM is None:
        _PROGRAM = _build_program()
    return _PROGRAM


def _masks():
    # msk[p, f] = 1 where f >= p (valid region of a diagonal 128-band)
    f = np.arange(P)[None, :]
    p = np.arange(P)[:, None]
    return (f >= p).astype(ml_dtypes.bfloat16)


def make_in_maps(x, w_qkv, w_proj):
    bf16 = ml_dtypes.bfloat16
    x = np.asarray(x, dtype=np.float32)
    w_qkv = np.asarray(w_qkv, dtype=np.float32)
    w_proj = np.asarray(w_proj, dtype=np.float32)
    wq, wk, wv = w_qkv[0:C], w_qkv[C:2 * C], w_qkv[2 * C:3 * C]
    msk = _masks()
    xTs = [np.ascontiguousarray(x[b].T.astype(bf16)) for b in range(B)]
    in_maps = []
    for core in range(NCORES):
        b, g = divmod(core, GROUPS)
        ds = slice(g * DQ, (g + 1) * DQ)
        in_maps.append(
            {
                "xt": xTs[b],
                "wqk": np.ascontiguousarray(
                    np.concatenate([wq[ds].T, wk[ds].T], axis=1).astype(bf16)
                ),
                "wv": np.ascontiguousarray(wv[ds].T.astype(bf16)),
                "wp": np.ascontiguousarray(w_proj[:, ds].T.astype(bf16)),
                "msk": msk,
            }
        )
    return in_maps


def kernel(x, w_qkv, w_proj):
    global LAST_RESULTS
    import os

    in_maps = make_in_maps(x, w_qkv, w_proj)
    nc = _get_program()
    try:
        res = run_bass_kernel_spmd(
            nc,
            in_maps,
            core_ids=list(range(NCORES)),
            trace=bool(os.environ.get("BASS_TRACE")),
        )
    except ModuleNotFoundError:
        # profiling hook unavailable in this environment; rerun untraced
        os.environ["BASS_NEVER_TRACE"] = "1"
        res = run_bass_kernel_spmd(nc, in_maps, core_ids=list(range(NCORES)))
    LAST_RESULTS = res
    out = np.zeros((B, T, C), dtype=np.float32)
    for core in range(NCORES):
        out[core // GROUPS] += np.asarray(res.results[core]["y"],
                                          dtype=np.float32)
    return out
